# revision 28
# baseline (speedup 1.0000x reference)
"""Trainium2 Bass kernel for GammaLambdaLearner lambda-return scan.

Computes, per batch row b (backward over time t = S-1 .. 0):

    gamma   = max(tanh(raw_gamma), 1e-8)            # scalar
    lambd_t = max(tanh(raw_lambd[t]), 1e-8)         # [S]
    ret[t]  = r[t] + gamma*(1-d[t])*((1-lambd_t)*v[t+1] + lambd_t*ret[t+1])
    ret[S]  := v[S]   (bootstrap carry)

which is the first-order linear recurrence ret[t] = b[t] + a[t]*ret[t+1] with

    a[t] = gamma*lambd_t*(1-d[t])
    b[t] = r[t] + gamma*(1-lambd_t)*(1-d[t])*v[t+1]

Mapping: batch is data-parallel across the 8 NeuronCores (1024 rows/core),
and within a core across the 128 SBUF partitions (8 row-tiles of
[128, 2048]).  Time lives in the free dimension; the recurrence runs on the
TensorTensorScan instruction (f32 carry) with reversed access patterns so
the backward order is free.

The kernel is DMA-byte-bound (~360 B/ns aggregate), so I/O is compressed:
v[t+1] and dones ride as fp8-e3m4 (dones are exactly 0/1; the v term
enters b with a ~0.1x gamma*(1-lambda) coefficient so fp8's ~3% rounding
is diluted), rewards and the output as bf16 (output upcast to f32 on the
host), and the scan carry stays f32 with the bootstrap v[S] column shipped
separately in bf16.  Host-side work is limited to dtype casts and layout
(slicing, replication, concatenation); all arithmetic runs on device.
Max rel err vs the f32 reference is ~5.5e-3 against the 2e-2 tolerance.

Engine split per [128, 2048] tile (the Pool/GPSIMD engine only supports
TensorTensor among the elementwise ops — TensorScalarPtr variants, scans
included, are DVE-only on real hardware):
  ACT    u  = 1 - d             (fp8 in, bf16 out; affine activation)
         w := bf16(v_fp8)       (upconvert so DVE runs its 2x bf16 mode)
  GPSIMD a  = u * glamR         (tensor_mul — the one legal Pool pass)
  DVE    w := u*w ; w := w*gmlamR ; w += r   (bf16 tensor_tensor, 2x mode)
  DVE    scan + chunked tile-0 / tile-7 (fill and drain pipelining)

Engine streams execute strictly in order and head-block on unsatisfied
semaphore waits, so cross-engine consumers are issued with a tile lag
(scan at lag 1, stores at lag 2) — by the time each instruction decodes
its wait is already satisfied.  Loads ride the SP HWDGE ring (tile 0's on
the ACT ring so they race the lambda-param load), stores the ACT ring.
The first and last tiles are column-chunked so the pipeline fill and the
final scan->store drain overlap with steady-state work.
"""

import numpy as np
import ml_dtypes

import concourse.bass as bass
import concourse.tile as tile
import concourse.mybir as mybir
from concourse import bacc
from concourse.bass_utils import run_bass_kernel_spmd

B, S = 8192, 2048
N_CORES = 8
R = B // N_CORES          # rows per core
P = 128                   # SBUF partitions
NT = R // P               # row-tiles per core
EPS = 1e-8

F32 = mybir.dt.float32
BF16 = mybir.dt.bfloat16
F8 = mybir.dt.float8e3
ALU = mybir.AluOpType

NP_BF16 = ml_dtypes.bfloat16
NP_F8 = ml_dtypes.float8_e3m4

# Load balance: which row-tiles scan on GPSIMD (DVE otherwise), and which
# compute their a-pass on DVE (GPSIMD otherwise).
A_GP_TILES = tuple(range(8))
ADD_MODE = "dve"   # "accum" | "dve" | "gp"
SCAN_LAG = 1


def build_kernel(rows=R, s=S, bufs=6, split_last=3, scan_lag=SCAN_LAG,
                 a_gp=A_GP_TILES, add_mode=ADD_MODE, add_gp=(), w_gp=(),
                 tile0_chunks=2, add_split=0, tmp_bufs=None, out_bufs=None):
    nt = rows // P
    nc = bacc.Bacc(
        "TRN2",
        target_bir_lowering=False,
        debug=False,
        enable_asserts=False,
        num_devices=N_CORES,
    )
    # vd packs v[:, 1:s+1] (cols 0..s-1) and dones (cols s..2s-1) in fp8
    vd = nc.dram_tensor("vd", [rows, 2 * s], F8, kind="ExternalInput").ap()
    rewards = nc.dram_tensor("rewards", [rows, s], BF16, kind="ExternalInput").ap()
    vboot = nc.dram_tensor("vboot", [rows, 1], BF16, kind="ExternalInput").ap()
    # raw_lambd replicated to all 128 partitions host-side so the one-time
    # param prep runs full-width instead of single-partition
    lam_rep = nc.dram_tensor("lam_rep", [P, s], BF16, kind="ExternalInput").ap()
    g_rep = nc.dram_tensor("g_rep", [P, 1], F32, kind="ExternalInput").ap()
    ret = nc.dram_tensor("ret", [rows, s], BF16, kind="ExternalOutput").ap()

    store_lag = scan_lag + 1

    with tile.TileContext(nc) as tc:
        with (
            tc.tile_pool(name="const", bufs=1) as const_pool,
            tc.tile_pool(name="ins", bufs=bufs) as in_pool,
            tc.tile_pool(name="tmp", bufs=tmp_bufs or (scan_lag + 2)) as tmp_pool,
            tc.tile_pool(name="out", bufs=out_bufs or (store_lag + 2)) as out_pool,
        ):
            state = [None] * nt  # per-tile (a, w, b, o, rs)

            # ---- loads for the params and tile 0 ----
            # lamR and tile 0's data race down separate rings so the lambda
            # prep and tile 0's ACT work start as early as possible
            g = const_pool.tile([P, 1], F32, tag="g")
            nc.scalar.dma_start(g[:], g_rep[:])
            lamR = const_pool.tile([P, s], BF16, tag="lamR")
            nc.sync.dma_start(lamR[:], lam_rep[:])
            vd0 = in_pool.tile([P, 2 * s], F8, tag="vd")
            nc.scalar.dma_start(vd0[:], vd[0:P, :])
            r0 = in_pool.tile([P, s], BF16, tag="r")
            nc.sync.dma_start(r0[:], rewards[0:P, :])
            vb = const_pool.tile([P, nt], BF16, tag="vb")
            # vboot dram rows map to (tile, partition); sbuf[p, i] = vboot[i*P+p]
            nc.scalar.dma_start(vb[:], vboot[:].rearrange("(i p) o -> p (i o)", p=P))

            # ---- parameter prep, high half first (the backward scan and the
            # chunked tile 0 consume high columns first) ----
            nc.scalar.activation(g[:], g[:], mybir.ActivationFunctionType.Tanh)
            nc.vector.tensor_scalar_max(g[:], g[:], EPS)
            glamR = lamR  # in place
            gmlamR = const_pool.tile([P, s], BF16, tag="gmlamR")
            for lo, hi in ((s // 2, s), (0, s // 2)):
                cs = slice(lo, hi)
                nc.scalar.activation(
                    lamR[:, cs], lamR[:, cs], mybir.ActivationFunctionType.Tanh
                )
                # glamR = max(tanh(lam), eps) * gamma
                nc.vector.tensor_scalar(
                    glamR[:, cs], lamR[:, cs], EPS, g[:, 0:1],
                    op0=ALU.max, op1=ALU.mult,
                )
                # gmlamR = gamma - glamR = gamma*(1-lambda)
                nc.vector.tensor_scalar(
                    gmlamR[:, cs], glamR[:, cs], -1.0, g[:, 0:1],
                    op0=ALU.mult, op1=ALU.add,
                )

            def emit_loads(i):
                rs = slice(i * P, (i + 1) * P)
                vd_t = in_pool.tile([P, 2 * s], F8, tag="vd")
                nc.sync.dma_start(vd_t[:], vd[rs, :])
                return rs, vd_t

            def chunk_bounds(n):
                if n == 7 and s % 16 == 0:
                    return [0, s // 16, s // 4, s]
                if n == 6 and s % 16 == 0:
                    return [0, s // 16, s // 4, s // 2, s]
                if n == 5 and s % 16 == 0:
                    return [0, s // 16, s // 8, s // 4, s // 2, s]
                if n == 4 and s % 8 == 0:
                    return [0, s // 8, s // 4, s // 2, s]
                if n == 3 and s % 4 == 0:
                    return [0, s // 4, s // 2, s]
                if n == 2:
                    return [0, s // 2, s]
                return [0, s]

            # ---- chunked tiles: column-chunked end to end so the scan
            # starts while later chunks are still being assembled (collapses
            # pipeline fill for tile 0 and the store drain for the last tile)
            def emit_tile_chunked(i, vd_t, r_t, bounds, store_now=False):
                rs = slice(i * P, (i + 1) * P)
                v_t = vd_t[:, 0:s]
                d_t = vd_t[:, s : 2 * s]
                u = tmp_pool.tile([P, s], BF16, tag="u")
                w = tmp_pool.tile([P, s], BF16, tag="w")
                a = tmp_pool.tile([P, s], BF16, tag="a")
                o = out_pool.tile([P, s], BF16, tag="o")
                for pc in range(len(bounds) - 2, -1, -1):
                    lo, hi = bounds[pc], bounds[pc + 1]
                    cs = slice(lo, hi)
                    nc.scalar.activation(
                        u[:, cs], d_t[:, cs], mybir.ActivationFunctionType.Copy,
                        bias=1.0, scale=-1.0,
                    )
                    nc.scalar.activation(
                        w[:, cs], v_t[:, cs], mybir.ActivationFunctionType.Copy
                    )
                    if i in a_gp:
                        nc.gpsimd.tensor_mul(a[:, cs], u[:, cs], glamR[:, cs])
                    else:
                        nc.vector.tensor_mul(a[:, cs], u[:, cs], glamR[:, cs])
                    nc.vector.tensor_mul(w[:, cs], u[:, cs], w[:, cs])
                    nc.vector.tensor_mul(w[:, cs], w[:, cs], gmlamR[:, cs])
                    nc.vector.tensor_add(w[:, cs], w[:, cs], r_t[:, cs])
                    init = vb[:, i : i + 1] if hi == s else o[:, hi : hi + 1]
                    nc.vector.tensor_tensor_scan(
                        o[:, cs][:, ::-1],
                        a[:, cs][:, ::-1],
                        w[:, cs][:, ::-1],
                        init,
                        op0=ALU.mult,
                        op1=ALU.add,
                    )
                    if store_now:
                        nc.scalar.dma_start(ret[rs, cs], o[:, cs])
                state[i] = (a, w, w, o, rs)

            def emit_front(i, rs, vd_t):
                v_t = vd_t[:, 0:s]
                d_t = vd_t[:, s : 2 * s]

                u = tmp_pool.tile([P, s], BF16, tag="u")
                w = tmp_pool.tile([P, s], BF16, tag="w")
                a = tmp_pool.tile([P, s], BF16, tag="a")
                o = out_pool.tile([P, s], BF16, tag="o")

                # u = 1 - d (exact in bf16) on ACT
                nc.scalar.activation(
                    u[:], d_t, mybir.ActivationFunctionType.Copy,
                    bias=1.0, scale=-1.0,
                )
                # a = u * glamR — TensorTensor is the one elementwise op the
                # Pool engine legally supports, so GPSIMD absorbs this pass
                if i in a_gp:
                    nc.gpsimd.tensor_mul(a[:], u[:], glamR[:])
                else:
                    nc.vector.tensor_mul(a[:], u[:], glamR[:])
                if i in w_gp:
                    # GPSIMD multiplies the raw fp8 v directly (dtype-blind),
                    # skipping the ACT upconvert for this tile; like the
                    # a-pass this only needs u, so Pool's stream stays
                    # dependency-shallow
                    nc.gpsimd.tensor_mul(w[:], u[:], v_t)
                else:
                    # w := bf16(v) on ACT so DVE runs its 2x bf16 mode
                    nc.scalar.activation(
                        w[:], v_t, mybir.ActivationFunctionType.Copy
                    )
                    nc.vector.tensor_mul(w[:], u[:], w[:])
                # w2 = (u * v) * gmlam — bf16 tensor_tensor (2x mode)
                nc.vector.tensor_mul(w[:], w[:], gmlamR[:])
                state[i] = (a, w, w, o, rs)

            def emit_accum(i):
                # b = w2 + r
                _, w, b, _, rs = state[i]
                if add_mode == "accum":
                    nc.gpsimd.dma_start(w[:], rewards[rs, :], accum_op=ALU.add)
                    return
                r_t = in_pool.tile([P, s], BF16, tag="r")
                nc.sync.dma_start(r_t[:], rewards[rs, :])
                if add_mode == "gp" or i in add_gp:
                    nc.gpsimd.tensor_add(w[:], w[:], r_t[:])
                elif add_split:
                    # low columns on GPSIMD, rest on DVE — the scan consumes
                    # high columns first, so the slow engine gets the slack
                    x = add_split
                    nc.gpsimd.tensor_add(w[:, 0:x], w[:, 0:x], r_t[:, 0:x])
                    nc.vector.tensor_add(w[:, x:], w[:, x:], r_t[:, x:])
                else:
                    nc.vector.tensor_add(w[:], w[:], r_t[:])

            def emit_back(i, bounds, store_now=False):
                a, w, b, o, rs = state[i]
                scan_eng = nc.vector
                for pc in range(len(bounds) - 2, -1, -1):
                    lo, hi = bounds[pc], bounds[pc + 1]
                    cs = slice(lo, hi)
                    # backward scan via reversed access patterns; carry chains
                    # from v[S] (top chunk) / previous chunk's first column
                    if hi == s:
                        init = vb[:, i : i + 1]
                    else:
                        init = o[:, hi : hi + 1]
                    scan_eng.tensor_tensor_scan(
                        o[:, cs][:, ::-1],
                        a[:, cs][:, ::-1],
                        b[:, cs][:, ::-1],
                        init,
                        op0=ALU.mult,
                        op1=ALU.add,
                    )
                    if store_now:
                        nc.scalar.dma_start(ret[rs, cs], o[:, cs])

            def emit_store(i):
                o, rs = state[i][3], state[i][4]
                nc.scalar.dma_start(ret[rs, :], o[:])

            emit_tile_chunked(0, vd0, r0, chunk_bounds(tile0_chunks))
            # stores ride the ACT ring two tiles late: by the time the
            # dma_start decodes, the scan has completed, so it never
            # head-blocks ACT's u/vb stream
            for i in range(1, nt):
                rs, vd_t = emit_loads(i)
                if i >= 2:
                    emit_accum(i - 1)
                emit_front(i, rs, vd_t)
                if i >= 2:
                    emit_back(i - 1, [0, s])
                if i >= 2:
                    emit_store(i - 2)
            emit_accum(nt - 1)
            emit_store(nt - 2)
            # tail: only the last tile's scan+store are chunked, so each
            # chunk's store overlaps the next chunk's scan (ACT is idle
            # by then — head-blocking is harmless)
            emit_back(nt - 1, chunk_bounds(split_last), store_now=True)

    nc.compile()
    return nc


_nc_cache = {}


def _get_nc():
    if "nc" not in _nc_cache:
        _nc_cache["nc"] = build_kernel()
    return _nc_cache["nc"]


def kernel(values, rewards, dones, raw_gamma, raw_lambd, trace=False):
    values = np.ascontiguousarray(values, np.float32).reshape(B, S + 1)
    rewards = np.ascontiguousarray(rewards, np.float32).reshape(B, S)
    dones = np.ascontiguousarray(dones, np.float32).reshape(B, S)

    # fp8 pack of v[t+1] | dones; bf16 rewards; bf16 bootstrap column
    vd = np.empty((B, 2 * S), dtype=NP_F8)
    vd[:, :S] = values[:, 1 : S + 1].astype(NP_F8)
    vd[:, S:] = dones.astype(NP_F8)
    r_q = rewards.astype(NP_BF16)
    vboot = np.ascontiguousarray(values[:, S : S + 1]).astype(NP_BF16)
    lam_rep = np.ascontiguousarray(
        np.broadcast_to(
            np.asarray(raw_lambd, np.float32).reshape(1, S).astype(NP_BF16), (P, S)
        )
    )
    g_rep = np.ascontiguousarray(
        np.broadcast_to(np.asarray(raw_gamma, np.float32).reshape(1, 1), (P, 1))
    )

    in_maps = []
    for c in range(N_CORES):
        rs = slice(c * R, (c + 1) * R)
        in_maps.append(
            {
                "vd": vd[rs],
                "rewards": r_q[rs],
                "vboot": vboot[rs],
                "lam_rep": lam_rep,
                "g_rep": g_rep,
            }
        )

    nc = _get_nc()
    if not trace:
        # NTFF profiling needs axon hooks that may be absent; force it off
        # unless explicitly requested
        import os

        os.environ["BASS_NEVER_TRACE"] = "1"
    try:
        res = run_bass_kernel_spmd(
            nc, in_maps, core_ids=list(range(N_CORES)), trace=trace
        )
    except Exception:
        # transient NRT/axon hiccups (e.g. a wedged exec unit from a prior
        # run) are recoverable on retry
        res = run_bass_kernel_spmd(
            nc, in_maps, core_ids=list(range(N_CORES)), trace=trace
        )
    out = np.concatenate(
        [np.asarray(res.results[c]["ret"]) for c in range(N_CORES)], axis=0
    )
    if trace:
        kernel.last_results = res
    return out.astype(np.float32).reshape(B, S, 1)


# revision 29
# speedup vs baseline: 1.0010x; 1.0010x over previous
"""Trainium2 Bass kernel for GammaLambdaLearner lambda-return scan.

Computes, per batch row b (backward over time t = S-1 .. 0):

    gamma   = max(tanh(raw_gamma), 1e-8)            # scalar
    lambd_t = max(tanh(raw_lambd[t]), 1e-8)         # [S]
    ret[t]  = r[t] + gamma*(1-d[t])*((1-lambd_t)*v[t+1] + lambd_t*ret[t+1])
    ret[S]  := v[S]   (bootstrap carry)

which is the first-order linear recurrence ret[t] = b[t] + a[t]*ret[t+1] with

    a[t] = gamma*lambd_t*(1-d[t])
    b[t] = r[t] + gamma*(1-lambd_t)*(1-d[t])*v[t+1]

Mapping: batch is data-parallel across the 8 NeuronCores (1024 rows/core),
and within a core across the 128 SBUF partitions (8 row-tiles of
[128, 2048]).  Time lives in the free dimension; the recurrence runs on the
TensorTensorScan instruction (f32 carry) with reversed access patterns so
the backward order is free.

The kernel is DMA-byte-bound (~360 B/ns aggregate), so I/O is compressed:
v[t+1] and dones ride as fp8-e3m4 (dones are exactly 0/1; the v term
enters b with a ~0.1x gamma*(1-lambda) coefficient so fp8's ~3% rounding
is diluted), rewards and the output as bf16 (output upcast to f32 on the
host), and the scan carry stays f32 with the bootstrap v[S] column shipped
separately in bf16.  Host-side work is limited to dtype casts and layout
(slicing, replication, concatenation); all arithmetic runs on device.
Max rel err vs the f32 reference is ~5.5e-3 against the 2e-2 tolerance.

Engine split per [128, 2048] tile (the Pool/GPSIMD engine only supports
TensorTensor among the elementwise ops — TensorScalarPtr variants, scans
included, are DVE-only on real hardware):
  ACT    u  = 1 - d             (fp8 in, bf16 out; affine activation)
         w := bf16(v_fp8)       (upconvert so DVE runs its 2x bf16 mode)
  GPSIMD a  = u * glamR         (tensor_mul — the one legal Pool pass)
  DVE    w := u*w ; w := w*gmlamR ; w += r   (bf16 tensor_tensor, 2x mode)
  DVE    scan + chunked tile-0 / tile-7 (fill and drain pipelining)

Engine streams execute strictly in order and head-block on unsatisfied
semaphore waits, so cross-engine consumers are issued with a tile lag
(scan at lag 1, stores at lag 2) — by the time each instruction decodes
its wait is already satisfied.  Loads ride the SP HWDGE ring (tile 0's on
the ACT ring so they race the lambda-param load), stores the ACT ring.
The first and last tiles are column-chunked so the pipeline fill and the
final scan->store drain overlap with steady-state work.
"""

import numpy as np
import ml_dtypes

import concourse.bass as bass
import concourse.tile as tile
import concourse.mybir as mybir
from concourse import bacc
from concourse.bass_utils import run_bass_kernel_spmd

B, S = 8192, 2048
N_CORES = 8
R = B // N_CORES          # rows per core
P = 128                   # SBUF partitions
NT = R // P               # row-tiles per core
EPS = 1e-8

F32 = mybir.dt.float32
BF16 = mybir.dt.bfloat16
F8 = mybir.dt.float8e3
ALU = mybir.AluOpType

NP_BF16 = ml_dtypes.bfloat16
NP_F8 = ml_dtypes.float8_e3m4

# Load balance: which row-tiles scan on GPSIMD (DVE otherwise), and which
# compute their a-pass on DVE (GPSIMD otherwise).
A_GP_TILES = tuple(range(8))
ADD_MODE = "dve"   # "accum" | "dve" | "gp"
SCAN_LAG = 1


def build_kernel(rows=R, s=S, bufs=6, split_last=3, scan_lag=SCAN_LAG,
                 a_gp=A_GP_TILES, add_mode=ADD_MODE, add_gp=(), w_gp=(),
                 tile0_chunks=2, add_split=0, tmp_bufs=None, out_bufs=None):
    nt = rows // P
    nc = bacc.Bacc(
        "TRN2",
        target_bir_lowering=False,
        debug=False,
        enable_asserts=False,
        num_devices=N_CORES,
    )
    # vd packs v[:, 1:s+1] (cols 0..s-1) and dones (cols s..2s-1) in fp8
    vd = nc.dram_tensor("vd", [rows, 2 * s], F8, kind="ExternalInput").ap()
    rewards = nc.dram_tensor("rewards", [rows, s], BF16, kind="ExternalInput").ap()
    vboot = nc.dram_tensor("vboot", [rows, 1], BF16, kind="ExternalInput").ap()
    # raw_lambd replicated to all 128 partitions host-side so the one-time
    # param prep runs full-width instead of single-partition
    lam_rep = nc.dram_tensor("lam_rep", [P, s], BF16, kind="ExternalInput").ap()
    g_rep = nc.dram_tensor("g_rep", [P, 1], F32, kind="ExternalInput").ap()
    ret = nc.dram_tensor("ret", [rows, s], BF16, kind="ExternalOutput").ap()

    store_lag = scan_lag + 1

    with tile.TileContext(nc) as tc:
        with (
            tc.tile_pool(name="const", bufs=1) as const_pool,
            tc.tile_pool(name="ins", bufs=bufs) as in_pool,
            tc.tile_pool(name="tmp", bufs=tmp_bufs or (scan_lag + 2)) as tmp_pool,
            tc.tile_pool(name="out", bufs=out_bufs or (store_lag + 2)) as out_pool,
        ):
            state = [None] * nt  # per-tile (a, w, b, o, rs)

            # ---- loads for the params and tile 0 ----
            # lamR and tile 0's data race down separate rings so the lambda
            # prep and tile 0's ACT work start as early as possible
            g = const_pool.tile([P, 1], F32, tag="g")
            nc.scalar.dma_start(g[:], g_rep[:])
            lamR = const_pool.tile([P, s], BF16, tag="lamR")
            nc.sync.dma_start(lamR[:], lam_rep[:])
            vd0 = in_pool.tile([P, 2 * s], F8, tag="vd")
            nc.scalar.dma_start(vd0[:], vd[0:P, :])
            r0 = in_pool.tile([P, s], BF16, tag="r")
            nc.sync.dma_start(r0[:], rewards[0:P, :])
            vb = const_pool.tile([P, nt], BF16, tag="vb")
            # vboot dram rows map to (tile, partition); sbuf[p, i] = vboot[i*P+p]
            nc.scalar.dma_start(vb[:], vboot[:].rearrange("(i p) o -> p (i o)", p=P))

            # ---- parameter prep, high half first (the backward scan and the
            # chunked tile 0 consume high columns first) ----
            nc.scalar.activation(g[:], g[:], mybir.ActivationFunctionType.Tanh)
            nc.vector.tensor_scalar_max(g[:], g[:], EPS)
            glamR = lamR  # in place
            gmlamR = const_pool.tile([P, s], BF16, tag="gmlamR")
            for lo, hi in ((s // 2, s), (0, s // 2)):
                cs = slice(lo, hi)
                nc.scalar.activation(
                    lamR[:, cs], lamR[:, cs], mybir.ActivationFunctionType.Tanh
                )
                # glamR = max(tanh(lam), eps) * gamma
                nc.vector.tensor_scalar(
                    glamR[:, cs], lamR[:, cs], EPS, g[:, 0:1],
                    op0=ALU.max, op1=ALU.mult,
                )
                # gmlamR = gamma - glamR = gamma*(1-lambda)
                nc.vector.tensor_scalar(
                    gmlamR[:, cs], glamR[:, cs], -1.0, g[:, 0:1],
                    op0=ALU.mult, op1=ALU.add,
                )

            def emit_loads(i):
                rs = slice(i * P, (i + 1) * P)
                vd_t = in_pool.tile([P, 2 * s], F8, tag="vd")
                nc.sync.dma_start(vd_t[:], vd[rs, :])
                return rs, vd_t

            def chunk_bounds(n):
                if n == 7 and s % 16 == 0:
                    return [0, s // 16, s // 4, s]
                if n == 6 and s % 16 == 0:
                    return [0, s // 16, s // 4, s // 2, s]
                if n == 5 and s % 16 == 0:
                    return [0, s // 16, s // 8, s // 4, s // 2, s]
                if n == 4 and s % 8 == 0:
                    return [0, s // 8, s // 4, s // 2, s]
                if n == 3 and s % 4 == 0:
                    return [0, s // 4, s // 2, s]
                if n == 2:
                    return [0, s // 2, s]
                return [0, s]

            # ---- chunked tiles: column-chunked end to end so the scan
            # starts while later chunks are still being assembled (collapses
            # pipeline fill for tile 0 and the store drain for the last tile)
            def emit_tile_chunked(i, vd_t, r_t, bounds, store_now=False):
                rs = slice(i * P, (i + 1) * P)
                v_t = vd_t[:, 0:s]
                d_t = vd_t[:, s : 2 * s]
                u = tmp_pool.tile([P, s], BF16, tag="u")
                w = tmp_pool.tile([P, s], BF16, tag="w")
                a = tmp_pool.tile([P, s], BF16, tag="a")
                o = out_pool.tile([P, s], BF16, tag="o")
                for pc in range(len(bounds) - 2, -1, -1):
                    lo, hi = bounds[pc], bounds[pc + 1]
                    cs = slice(lo, hi)
                    nc.scalar.activation(
                        u[:, cs], d_t[:, cs], mybir.ActivationFunctionType.Copy,
                        bias=1.0, scale=-1.0,
                    )
                    nc.scalar.activation(
                        w[:, cs], v_t[:, cs], mybir.ActivationFunctionType.Copy
                    )
                    if i in a_gp:
                        nc.gpsimd.tensor_mul(a[:, cs], u[:, cs], glamR[:, cs])
                    else:
                        nc.vector.tensor_mul(a[:, cs], u[:, cs], glamR[:, cs])
                    nc.vector.tensor_mul(w[:, cs], u[:, cs], w[:, cs])
                    nc.vector.tensor_mul(w[:, cs], w[:, cs], gmlamR[:, cs])
                    nc.vector.tensor_add(w[:, cs], w[:, cs], r_t[:, cs])
                    init = vb[:, i : i + 1] if hi == s else o[:, hi : hi + 1]
                    nc.vector.tensor_tensor_scan(
                        o[:, cs][:, ::-1],
                        a[:, cs][:, ::-1],
                        w[:, cs][:, ::-1],
                        init,
                        op0=ALU.mult,
                        op1=ALU.add,
                    )
                    if store_now:
                        nc.scalar.dma_start(ret[rs, cs], o[:, cs])
                state[i] = (a, w, w, o, rs)

            def emit_front(i, rs, vd_t):
                v_t = vd_t[:, 0:s]
                d_t = vd_t[:, s : 2 * s]

                u = tmp_pool.tile([P, s], BF16, tag="u")
                w = tmp_pool.tile([P, s], BF16, tag="w")
                a = tmp_pool.tile([P, s], BF16, tag="a")
                o = out_pool.tile([P, s], BF16, tag="o")

                # u = 1 - d (exact in bf16) on ACT
                nc.scalar.activation(
                    u[:], d_t, mybir.ActivationFunctionType.Copy,
                    bias=1.0, scale=-1.0,
                )
                # a = u * glamR — TensorTensor is the one elementwise op the
                # Pool engine legally supports, so GPSIMD absorbs this pass
                if i in a_gp:
                    nc.gpsimd.tensor_mul(a[:], u[:], glamR[:])
                else:
                    nc.vector.tensor_mul(a[:], u[:], glamR[:])
                if i in w_gp:
                    # GPSIMD multiplies the raw fp8 v directly (dtype-blind),
                    # skipping the ACT upconvert for this tile; like the
                    # a-pass this only needs u, so Pool's stream stays
                    # dependency-shallow
                    nc.gpsimd.tensor_mul(w[:], u[:], v_t)
                else:
                    # w := bf16(v) on ACT so DVE runs its 2x bf16 mode
                    nc.scalar.activation(
                        w[:], v_t, mybir.ActivationFunctionType.Copy
                    )
                    nc.vector.tensor_mul(w[:], u[:], w[:])
                # w2 = (u * v) * gmlam — bf16 tensor_tensor (2x mode)
                nc.vector.tensor_mul(w[:], w[:], gmlamR[:])
                state[i] = (a, w, w, o, rs)

            def emit_accum(i):
                # b = w2 + r
                _, w, b, _, rs = state[i]
                if add_mode == "accum":
                    nc.gpsimd.dma_start(w[:], rewards[rs, :], accum_op=ALU.add)
                    return
                r_t = in_pool.tile([P, s], BF16, tag="r")
                nc.sync.dma_start(r_t[:], rewards[rs, :])
                if add_mode == "gp" or i in add_gp:
                    nc.gpsimd.tensor_add(w[:], w[:], r_t[:])
                elif add_split:
                    # low columns on GPSIMD, rest on DVE — the scan consumes
                    # high columns first, so the slow engine gets the slack
                    x = add_split
                    nc.gpsimd.tensor_add(w[:, 0:x], w[:, 0:x], r_t[:, 0:x])
                    nc.vector.tensor_add(w[:, x:], w[:, x:], r_t[:, x:])
                else:
                    nc.vector.tensor_add(w[:], w[:], r_t[:])

            def emit_back(i, bounds, store_now=False):
                a, w, b, o, rs = state[i]
                scan_eng = nc.vector
                for pc in range(len(bounds) - 2, -1, -1):
                    lo, hi = bounds[pc], bounds[pc + 1]
                    cs = slice(lo, hi)
                    # backward scan via reversed access patterns; carry chains
                    # from v[S] (top chunk) / previous chunk's first column
                    if hi == s:
                        init = vb[:, i : i + 1]
                    else:
                        init = o[:, hi : hi + 1]
                    scan_eng.tensor_tensor_scan(
                        o[:, cs][:, ::-1],
                        a[:, cs][:, ::-1],
                        b[:, cs][:, ::-1],
                        init,
                        op0=ALU.mult,
                        op1=ALU.add,
                    )
                    if store_now:
                        nc.scalar.dma_start(ret[rs, cs], o[:, cs])

            def emit_store(i):
                o, rs = state[i][3], state[i][4]
                nc.scalar.dma_start(ret[rs, :], o[:])

            emit_tile_chunked(0, vd0, r0, chunk_bounds(tile0_chunks))
            # stores ride the ACT ring two tiles late: by the time the
            # dma_start decodes, the scan has completed, so it never
            # head-blocks ACT's u/vb stream
            for i in range(1, nt):
                # virtual-time pin at the steady-state pipeline rhythm
                # (~5.6us/tile) — keeps the tile scheduler's greedy choices
                # from drifting off the intended cadence
                tc.tile_set_cur_wait(0.004 + 0.0056 * (i - 1))
                rs, vd_t = emit_loads(i)
                if i >= 2:
                    emit_accum(i - 1)
                emit_front(i, rs, vd_t)
                if i >= 2:
                    emit_back(i - 1, [0, s])
                if i >= 2:
                    emit_store(i - 2)
            emit_accum(nt - 1)
            emit_store(nt - 2)
            # tail: only the last tile's scan+store are chunked, so each
            # chunk's store overlaps the next chunk's scan (ACT is idle
            # by then — head-blocking is harmless)
            emit_back(nt - 1, chunk_bounds(split_last), store_now=True)

    nc.compile()
    return nc


_nc_cache = {}


def _get_nc():
    if "nc" not in _nc_cache:
        _nc_cache["nc"] = build_kernel()
    return _nc_cache["nc"]


def kernel(values, rewards, dones, raw_gamma, raw_lambd, trace=False):
    values = np.ascontiguousarray(values, np.float32).reshape(B, S + 1)
    rewards = np.ascontiguousarray(rewards, np.float32).reshape(B, S)
    dones = np.ascontiguousarray(dones, np.float32).reshape(B, S)

    # fp8 pack of v[t+1] | dones; bf16 rewards; bf16 bootstrap column
    vd = np.empty((B, 2 * S), dtype=NP_F8)
    vd[:, :S] = values[:, 1 : S + 1].astype(NP_F8)
    vd[:, S:] = dones.astype(NP_F8)
    r_q = rewards.astype(NP_BF16)
    vboot = np.ascontiguousarray(values[:, S : S + 1]).astype(NP_BF16)
    lam_rep = np.ascontiguousarray(
        np.broadcast_to(
            np.asarray(raw_lambd, np.float32).reshape(1, S).astype(NP_BF16), (P, S)
        )
    )
    g_rep = np.ascontiguousarray(
        np.broadcast_to(np.asarray(raw_gamma, np.float32).reshape(1, 1), (P, 1))
    )

    in_maps = []
    for c in range(N_CORES):
        rs = slice(c * R, (c + 1) * R)
        in_maps.append(
            {
                "vd": vd[rs],
                "rewards": r_q[rs],
                "vboot": vboot[rs],
                "lam_rep": lam_rep,
                "g_rep": g_rep,
            }
        )

    nc = _get_nc()
    if not trace:
        # NTFF profiling needs axon hooks that may be absent; force it off
        # unless explicitly requested
        import os

        os.environ["BASS_NEVER_TRACE"] = "1"
    try:
        res = run_bass_kernel_spmd(
            nc, in_maps, core_ids=list(range(N_CORES)), trace=trace
        )
    except Exception:
        # transient NRT/axon hiccups (e.g. a wedged exec unit from a prior
        # run) are recoverable on retry
        res = run_bass_kernel_spmd(
            nc, in_maps, core_ids=list(range(N_CORES)), trace=trace
        )
    out = np.concatenate(
        [np.asarray(res.results[c]["ret"]) for c in range(N_CORES)], axis=0
    )
    if trace:
        kernel.last_results = res
    return out.astype(np.float32).reshape(B, S, 1)
